# revision 9
# baseline (speedup 1.0000x reference)
"""Trainium2 kernel for the bilinear form y[b,k] = sum_ij x[b,i] x[b,j] W[i,j,k] + b[k].

Shapes: x (512, 784) f32, W (614656=784*784, 10) f32, b (10,) f32 -> y (512, 10) f32.

Strategy (8 NeuronCores):
  - Shard the j axis of W.reshape(784, 784, 10) across cores: 98 j's per core.
    Each core reads W/8 + full x (~2.5 MB in fp16); compute is the long pole.
  - Stage 1 (TensorE): U[b, (k,j)] = sum_i x[b,i] * W[i, j_shard, k], x^T tiles
    stationary, W shard moving, accumulating over 7 uniform 112-row i-tiles
    into 8 PSUM banks (4 batch tiles x 2 column halves = 5 k's x 98 j's).
  - Stage 2 (DVE, fused): tensor_tensor_reduce does
    y[b,k] = sum_j U[b,(k,j)] * x[b,j] in ONE pass per group.
  - Host: y = sum_c y_part_c + b  (20 KB per core; no collectives needed).

Schedule (calibrated against HW traces):
  - 3 DMA rings in parallel from t~1.4us: sync=W.h0, scalar=W.h1, gpsimd=xT+xs.
    First chunks (1 i-tile) land ~4.6us; later chunks 2-4 i-tiles wide.
  - Dummy-warmup matmuls from ~1.7us keep the PE continuously busy so the
    HAM clock boost (half->full PE clock, ~6.3us after first PE activity)
    arrives as early as possible.
  - P1: (it0, h0) then (it0, h1) across all 4 batch tiles (8 matmuls riding
    the slow-clock window). P2: group-major finish (its 1-6 per group), so
    groups close every ~1.2us and stage-2 ttr overlaps the matmul stream.
  - y leaves in 4 group-pair DMAs on the gpsimd ring (kept warm by the
    input DMAs + earlier y chunks), so the last chunk avoids the ~1.6us
    cold-ring restart latency.
"""

import numpy as np

D = 784
B = 512
C = 10
NCORES = 8
JS = D // NCORES  # 98 j's per core
JK = JS * C  # 980 free columns per core, laid out as (k, j)
HALF = JK // 2  # 490 = 5 k's x 98 j's -> one PSUM bank
KH = C // 2  # 5 k's per half
P = 128
B_TILES = B // P  # 4
IT = 7  # i-tiles
IP = D // IT  # 112 rows per i-tile (uniform, no padding)
N_WARMUP_MM = 8  # dummy matmuls riding the pre-data slow-clock window

MM_DTYPE = "float16"  # dtype of the matmul operands (and their DMA)

_nc_cache = {}


def _build_nc():
    import concourse.bacc as bacc
    import concourse.mybir as mybir
    import concourse.tile as tile

    mm_dt = getattr(mybir.dt, MM_DTYPE)
    f32 = mybir.dt.float32

    nc = bacc.Bacc("TRN2", target_bir_lowering=False)

    # Partition-major DRAM layouts (see _make_in_maps).
    xT = nc.dram_tensor("xT", [IP, IT, B], mm_dt, kind="ExternalInput")
    w = nc.dram_tensor("w", [IP, 2, IT, HALF], mm_dt, kind="ExternalInput")
    xs = nc.dram_tensor("xs", [P, B_TILES, JS], mm_dt, kind="ExternalInput")
    y = nc.dram_tensor("y", [P, 2, B_TILES, KH], f32, kind="ExternalOutput")

    with tile.TileContext(nc) as tc:
        with (
            tc.tile_pool(name="wpool", bufs=8) as wpool,
            tc.tile_pool(name="xpool", bufs=3) as xpool,
            tc.tile_pool(name="xspool", bufs=1) as xspool,
            tc.tile_pool(name="ypool", bufs=1) as ypool,
            tc.tile_pool(name="scratch", bufs=4) as spool,
            tc.tile_pool(name="psum", bufs=8, space="PSUM") as psum_pool,
        ):
            w_sb = {}  # (it, h) -> [IP, HALF] view

            def w_dma(eng, h, c0, c1):
                wt = wpool.tile(
                    [IP, c1 - c0, HALF], mm_dt, name=f"w_h{h}c{c0}", tag=f"wh{h}"
                )
                eng.dma_start(wt[:], w[:, h, c0:c1, :])
                for it in range(c0, c1):
                    w_sb[(it, h)] = wt[:, it - c0, :]

            xT_sb = {}

            def xt_dma(c0, c1):
                xt = xpool.tile([IP, c1 - c0, B], mm_dt, name=f"xt_c{c0}", tag="xt")
                nc.scalar.dma_start(xt[:], xT[:, c0:c1, :])
                for it in range(c0, c1):
                    xT_sb[it] = xt[:, it - c0, :]

            # Queue ops cost ~650ns each regardless of size: narrow 1-i-tile
            # first chunks for earliest unblock, fat chunks behind them.
            # sync: wh0[0:1], wh1[0:1], wh0[1:4], wh0[4:7]
            # scalar: xt[0:1], xt[1:3], xt[3:7], wh1[1:7]
            # gpsimd (SWDGE): xs, later the y pairs (ring stays warm)
            dmy_s = spool.tile([IP, P], mm_dt, name="dmy_s", tag="dmy_s", bufs=1)
            dmy_m = spool.tile([IP, HALF], mm_dt, name="dmy_m", tag="dmy_m", bufs=1)
            nc.gpsimd.memset(dmy_s[:], 0.0)
            nc.gpsimd.memset(dmy_m[:], 0.0)
            w_dma(nc.sync, 0, 0, 1)
            xt_dma(0, 1)
            w_dma(nc.sync, 1, 0, 1)
            xt_dma(1, 3)
            w_dma(nc.sync, 0, 1, 4)
            xt_dma(3, 7)
            w_dma(nc.sync, 0, 4, 7)
            w_dma(nc.scalar, 1, 1, 7)
            xs_sb = xspool.tile([P, B_TILES, JS], mm_dt)
            nc.gpsimd.dma_start(xs_sb[:], xs[:])

            # PSUM: 8 accumulation groups (bt, h), one bank each. Warmups
            # write into group (0,0)'s bank; the first real start=True matmul
            # clears has_written so the garbage is discarded.
            pts = {}
            for bt in range(B_TILES):
                for h in range(2):
                    pts[(bt, h)] = psum_pool.tile(
                        [P, HALF], f32, name=f"pt_b{bt}h{h}", tag="pt", bufs=8
                    )

            def warmup(n):
                # dummy matmuls into g00's bank (g00 is opened last, in P2);
                # each one opens and closes its own accumulation group.
                for _ in range(n):
                    nc.tensor.matmul(
                        pts[(0, 0)][:], dmy_s[:], dmy_m[:], start=True, stop=True
                    )

            warmup(N_WARMUP_MM)

            y_t = ypool.tile([P, 2, B_TILES, KH], f32)

            def mm(it, bt, h, start, stop):
                nc.tensor.matmul(
                    pts[(bt, h)][:],
                    xT_sb[it][:, bt * P : (bt + 1) * P],
                    w_sb[(it, h)][:],
                    start=start,
                    stop=stop,
                )

            def stage2(bt, h):
                # Multiply on DVE (PSUM read); reduce over j alternates
                # DVE / GpSimd so neither engine backs up near stream end.
                pt = pts[(bt, h)]
                scr = spool.tile(
                    [P, HALF], mm_dt, name=f"scr{bt}{h}", tag="scr", bufs=2
                )
                s3 = scr[:].rearrange("p (kh j) -> p kh j", kh=KH)
                p3 = pt[:].rearrange("p (kh j) -> p kh j", kh=KH)
                xs3 = xs_sb[:, bt, None, :].broadcast_to([P, KH, JS])
                nc.vector.tensor_tensor(s3, p3, xs3, mybir.AluOpType.mult)
                nc.vector.tensor_reduce(
                    out=y_t[:, h, bt, :],
                    in_=s3,
                    op=mybir.AluOpType.add,
                    axis=mybir.AxisListType.X,
                )

            # P1: it0 for every group except g00 (needs only the narrow
            # first chunks), then 2 cushion warmups into g00's untouched bank
            # (keeps the PE gap-free if the fat chunks run late).
            for bt in range(1, B_TILES):
                mm(0, bt, 0, start=True, stop=False)
            for bt in range(B_TILES):
                mm(0, bt, 1, start=True, stop=False)
            warmup(2)

            # P2: strict group-major; groups close every ~1.2us so the DVE
            # stage-2 (TT + fp16 reduce ~0.95us) never backs up; paired y
            # DMAs ride the warm gpsimd ring.
            order = [(bt, 0) for bt in range(B_TILES)] + [
                (bt, 1) for bt in range(B_TILES)
            ]
            for bt, h in order:
                it0 = 0 if (bt, h) == (0, 0) else 1
                for it in range(it0, IT):
                    mm(it, bt, h, start=(it == 0), stop=(it == IT - 1))
                stage2(bt, h)
                if bt % 2 == 1:
                    # ship groups (bt-1, h) and (bt, h) together
                    nc.gpsimd.dma_start(
                        y[:, h, bt - 1 : bt + 1, :], y_t[:, h, bt - 1 : bt + 1, :]
                    )

    nc.compile()
    return nc


def _get_nc():
    if "nc" not in _nc_cache:
        _nc_cache["nc"] = _build_nc()
    return _nc_cache["nc"]


def _make_in_maps(x, W):
    import concourse.mybir as mybir

    mm_np = mybir.dt.np(getattr(mybir.dt, MM_DTYPE))
    x = np.asarray(x, dtype=np.float32)
    Wr = np.asarray(W, dtype=np.float32).reshape(D, D, C)
    # xT_dram[p, it, b] = x[b, it*IP + p]
    xT = np.ascontiguousarray(
        x.T.astype(mm_np).reshape(IT, IP, B).transpose(1, 0, 2)
    )
    in_maps = []
    for c in range(NCORES):
        js, je = c * JS, (c + 1) * JS
        # wsh[i, k*JS + j] = Wr[i, js+j, k]; then [p, h, it, col] partition-major
        wsh = Wr[:, js:je, :].transpose(0, 2, 1).reshape(D, JK).astype(mm_np)
        wshard = np.ascontiguousarray(
            wsh.reshape(IT, IP, 2, HALF).transpose(1, 2, 0, 3)
        )
        # xs_dram[p, bt, j] = x[bt*P + p, js + j]
        xsl = np.ascontiguousarray(
            x[:, js:je].reshape(B_TILES, P, JS).transpose(1, 0, 2).astype(mm_np)
        )
        in_maps.append({"xT": xT, "w": wshard, "xs": xsl})
    return in_maps


def run_spmd(x, W, **spmd_kwargs):
    """Compile/run the SPMD kernel; returns (partials, BassKernelResults)."""
    from concourse.bass_utils import run_bass_kernel_spmd

    nc = _get_nc()
    in_maps = _make_in_maps(x, W)
    res = run_bass_kernel_spmd(nc, in_maps, core_ids=list(range(NCORES)), **spmd_kwargs)
    # y_dram[p, h, bt, kh] -> y[bt*P + p, h*KH + kh]
    partials = [
        r["y"].transpose(2, 0, 1, 3).reshape(B_TILES, P, C).reshape(B, C)
        for r in res.results
    ]
    return partials, res


def kernel(x, W, b):
    partials, _ = run_spmd(x, W)
    y = np.sum(np.stack(partials, 0), axis=0, dtype=np.float64) + np.asarray(
        b, dtype=np.float64
    )
    return y.astype(np.float32)


# revision 10
# speedup vs baseline: 1.2076x; 1.2076x over previous
"""Trainium2 kernel for the bilinear form y[b,k] = sum_ij x[b,i] x[b,j] W[i,j,k] + b[k].

Shapes: x (512, 784) f32, W (614656=784*784, 10) f32, b (10,) f32 -> y (512, 10) f32.

Strategy (8 NeuronCores):
  - Shard the j axis of W.reshape(784, 784, 10) across cores: 98 j's per core.
    Each core reads W/8 + full x (~2.5 MB in fp16); compute is the long pole.
  - Stage 1 (TensorE): U[b, (k,j)] = sum_i x[b,i] * W[i, j_shard, k], x^T tiles
    stationary, W shard moving, accumulating over 7 uniform 112-row i-tiles
    into 8 PSUM banks (4 batch tiles x 2 column halves = 5 k's x 98 j's).
  - Stage 2 (DVE): U * xs multiply (PSUM read) + reduce over j, ~1.14us per
    group; groups must close >=1.2us apart or the DVE backs up.
  - Host: y = sum_c y_part_c + b  (20 KB per core; no collectives needed).

Schedule (calibrated against HW traces):
  - First input chunk completes ~5.6us after kernel start (1.6us framework
    entry + 0.7us queue op + ~3.3us ring latency); total input delivery runs
    at ~0.22-0.3 MB/us; y DMA completion costs a fixed ~1.9us after issue;
    framework teardown is a fixed ~8.5us.  The PE runs at half clock until
    the HAM boost, ~6.3us of GAP-FREE PE activity after the first matmul;
    any PE idle gap resets the ramp (costing ~5us) -- so dummy warmup
    matmuls bridge from t~2.1 until real data lands.
  - xT is laid out bt-major so each batch-tile's stationary set (0.2 MB)
    arrives independently: the first group closes at ~9.5us instead of ~14.
  - Matmul order chases delivery group-major ((bt,h) columns of work), so
    the 8 PSUM groups close every ~1.4us and all but the last DVE stage-2
    hide under the matmul stream.
  - y leaves in 4 group-pair DMAs on the gpsimd ring as pairs complete.
"""

import numpy as np

D = 784
B = 512
C = 10
NCORES = 8
JS = D // NCORES  # 98 j's per core
JK = JS * C  # 980 free columns per core, laid out as (k, j)
HALF = JK // 2  # 490 = 5 k's x 98 j's -> one PSUM bank
KH = C // 2  # 5 k's per half
P = 128
B_TILES = B // P  # 4
IT = 7  # i-tiles
IP = D // IT  # 112 rows per i-tile (uniform, no padding)
N_WARMUP_MM = 10  # dummy matmuls bridging until the first chunks land

MM_DTYPE = "float16"  # dtype of the matmul operands (and their DMA)

_nc_cache = {}


def _build_nc():
    import concourse.bacc as bacc
    import concourse.mybir as mybir
    import concourse.tile as tile

    mm_dt = getattr(mybir.dt, MM_DTYPE)
    f32 = mybir.dt.float32

    nc = bacc.Bacc("TRN2", target_bir_lowering=False)

    # Partition-major DRAM layouts (see _make_in_maps).
    xT = nc.dram_tensor("xT", [IP, B_TILES, IT, P], mm_dt, kind="ExternalInput")
    w = nc.dram_tensor("w", [IP, 2, IT, HALF], mm_dt, kind="ExternalInput")
    xs = nc.dram_tensor("xs", [P, B_TILES, JS], mm_dt, kind="ExternalInput")
    y = nc.dram_tensor("y", [P, 2, B_TILES, KH], f32, kind="ExternalOutput")

    with tile.TileContext(nc) as tc:
        with (
            tc.tile_pool(name="wpool", bufs=8) as wpool,
            tc.tile_pool(name="xpool", bufs=4) as xpool,
            tc.tile_pool(name="xspool", bufs=1) as xspool,
            tc.tile_pool(name="ypool", bufs=1) as ypool,
            tc.tile_pool(name="scratch", bufs=4) as spool,
            tc.tile_pool(name="psum", bufs=8, space="PSUM") as psum_pool,
        ):
            # Dummy warmup operands; memset on gpsimd right after entry so
            # the first warmup matmul (and the HAM ramp clock) starts ASAP.
            dmy_s = spool.tile([IP, P], mm_dt, name="dmy_s", tag="dmy_s", bufs=1)
            dmy_m = spool.tile([IP, HALF], mm_dt, name="dmy_m", tag="dmy_m", bufs=1)
            nc.gpsimd.memset(dmy_s[:], 0.0)
            nc.gpsimd.memset(dmy_m[:], 0.0)

            w_sb = {}  # (it, h) -> [IP, HALF] view

            def w_dma(eng, h, c0, c1):
                wt = wpool.tile(
                    [IP, c1 - c0, HALF], mm_dt, name=f"w_h{h}c{c0}", tag=f"wh{h}"
                )
                eng.dma_start(wt[:], w[:, h, c0:c1, :])
                for it in range(c0, c1):
                    w_sb[(it, h)] = wt[:, it - c0, :]

            xT_sb = {}  # bt -> [IP, IT, P]

            def xt_dma(bt):
                xt = xpool.tile([IP, IT, P], mm_dt, name=f"xt_b{bt}", tag="xt")
                nc.scalar.dma_start(xt[:], xT[:, bt])
                xT_sb[bt] = xt

            # Issue order == per-ring delivery order (queue ops ~650ns each).
            # sync: wh0[0:2], wh0[2:4], wh0[4:7], wh1[4:7]
            # scalar: xt-bt0..3, wh1[0:2], wh1[2:4]
            # gpsimd: xs, then the y pairs (ring stays warm)
            w_dma(nc.sync, 0, 0, 2)
            xt_dma(0)
            xt_dma(1)
            w_dma(nc.sync, 0, 2, 4)
            xt_dma(2)
            w_dma(nc.sync, 0, 4, 7)
            xt_dma(3)
            w_dma(nc.sync, 1, 4, 7)
            w_dma(nc.scalar, 1, 0, 2)
            w_dma(nc.scalar, 1, 2, 4)
            xs_sb = xspool.tile([P, B_TILES, JS], mm_dt)
            nc.gpsimd.dma_start(xs_sb[:], xs[:])

            # PSUM: 8 accumulation groups (bt, h), one bank each.
            pts = {}
            for bt in range(B_TILES):
                for h in range(2):
                    pts[(bt, h)] = psum_pool.tile(
                        [P, HALF], f32, name=f"pt_b{bt}h{h}", tag="pt", bufs=8
                    )

            def warmup(n, bank=(3, 1)):
                # dummy matmuls into a not-yet-opened group's bank; each one
                # opens and closes its own accumulation group.
                for _ in range(n):
                    nc.tensor.matmul(
                        pts[bank][:], dmy_s[:], dmy_m[:], start=True, stop=True
                    )

            warmup(N_WARMUP_MM)

            y_t = ypool.tile([P, 2, B_TILES, KH], f32)

            def mm(it, bt, h):
                nc.tensor.matmul(
                    pts[(bt, h)][:],
                    xT_sb[bt][:, it, :],
                    w_sb[(it, h)][:],
                    start=(it == 0),
                    stop=(it == IT - 1),
                )

            def stage2(bt, h):
                # Multiply on DVE (PSUM read), then reduce over j on DVE.
                pt = pts[(bt, h)]
                scr = spool.tile(
                    [P, HALF], f32, name=f"scr{bt}{h}", tag="scr", bufs=2
                )
                s3 = scr[:].rearrange("p (kh j) -> p kh j", kh=KH)
                p3 = pt[:].rearrange("p (kh j) -> p kh j", kh=KH)
                xs3 = xs_sb[:, bt, None, :].broadcast_to([P, KH, JS])
                nc.vector.tensor_tensor(s3, p3, xs3, mybir.AluOpType.mult)
                nc.vector.tensor_reduce(
                    out=y_t[:, h, bt, :],
                    in_=s3,
                    op=mybir.AluOpType.add,
                    axis=mybir.AxisListType.X,
                )

            def y_dma(h, bt):
                # ship groups (bt-1, h) and (bt, h) together
                nc.gpsimd.dma_start(
                    y[:, h, bt - 1 : bt + 1, :], y_t[:, h, bt - 1 : bt + 1, :]
                )

            # Group-chasing stream: each step is a run of i-tiles for one
            # (bt, h) group, ordered so every run's W/xT chunks have landed
            # and the 8 closures spread ~1.4us apart.
            SCHED = [
                (0, 0, 0, 2), (1, 0, 0, 2),   # g00 g10 its0-1
                (0, 0, 2, 4), (1, 0, 2, 4),   # g00 g10 its2-3
                (0, 0, 4, 7),                 # g00 close c1
                (2, 0, 0, 2),                 # g20 its0-1
                (1, 0, 4, 7),                 # g10 close c2
                (2, 0, 2, 4), (3, 0, 0, 2),   # g20 its2-3, g30 its0-1
                (2, 0, 4, 7),                 # g20 close c3
                (3, 0, 2, 4), (0, 1, 0, 2),   # g30 its2-3, g01 its0-1
                (3, 0, 4, 7),                 # g30 close c4
                (0, 1, 2, 4), (1, 1, 0, 2),   # g01 its2-3, g11 its0-1
                (0, 1, 4, 7),                 # g01 close c5
                (1, 1, 2, 4), (2, 1, 0, 2),   # g11 its2-3, g21 its0-1
                (1, 1, 4, 7),                 # g11 close c6
                (2, 1, 2, 4), (3, 1, 0, 2),   # g21 its2-3, g31 its0-1
                (2, 1, 4, 7),                 # g21 close c7
                (3, 1, 2, 7),                 # g31 its2-6 close c8
            ]
            for bt, h, i0, i1 in SCHED:
                for it in range(i0, i1):
                    mm(it, bt, h)
                if i1 == IT:
                    stage2(bt, h)
                    if bt % 2 == 1:
                        y_dma(h, bt)

    nc.compile()
    return nc


def _get_nc():
    if "nc" not in _nc_cache:
        _nc_cache["nc"] = _build_nc()
    return _nc_cache["nc"]


def _make_in_maps(x, W):
    import concourse.mybir as mybir

    mm_np = mybir.dt.np(getattr(mybir.dt, MM_DTYPE))
    x = np.asarray(x, dtype=np.float32)
    Wr = np.asarray(W, dtype=np.float32).reshape(D, D, C)
    # xT_dram[p, bt, it, q] = x[bt*P + q, it*IP + p]
    xT = np.ascontiguousarray(
        x.T.astype(mm_np).reshape(IT, IP, B_TILES, P).transpose(1, 2, 0, 3)
    )
    in_maps = []
    for c in range(NCORES):
        js, je = c * JS, (c + 1) * JS
        # wsh[i, k*JS + j] = Wr[i, js+j, k]; then [p, h, it, col] partition-major
        wsh = Wr[:, js:je, :].transpose(0, 2, 1).reshape(D, JK).astype(mm_np)
        wshard = np.ascontiguousarray(
            wsh.reshape(IT, IP, 2, HALF).transpose(1, 2, 0, 3)
        )
        # xs_dram[p, bt, j] = x[bt*P + p, js + j]
        xsl = np.ascontiguousarray(
            x[:, js:je].reshape(B_TILES, P, JS).transpose(1, 0, 2).astype(mm_np)
        )
        in_maps.append({"xT": xT, "w": wshard, "xs": xsl})
    return in_maps


def run_spmd(x, W, **spmd_kwargs):
    """Compile/run the SPMD kernel; returns (partials, BassKernelResults)."""
    from concourse.bass_utils import run_bass_kernel_spmd

    nc = _get_nc()
    in_maps = _make_in_maps(x, W)
    res = run_bass_kernel_spmd(nc, in_maps, core_ids=list(range(NCORES)), **spmd_kwargs)
    # y_dram[p, h, bt, kh] -> y[bt*P + p, h*KH + kh]
    partials = [
        r["y"].transpose(2, 0, 1, 3).reshape(B_TILES, P, C).reshape(B, C)
        for r in res.results
    ]
    return partials, res


def kernel(x, W, b):
    partials, _ = run_spmd(x, W)
    y = np.sum(np.stack(partials, 0), axis=0, dtype=np.float64) + np.asarray(
        b, dtype=np.float64
    )
    return y.astype(np.float32)


# revision 11
# speedup vs baseline: 1.2626x; 1.0455x over previous
"""Trainium2 kernel for the bilinear form y[b,k] = sum_ij x[b,i] x[b,j] W[i,j,k] + b[k].

Shapes: x (512, 784) f32, W (614656=784*784, 10) f32, b (10,) f32 -> y (512, 10) f32.

Strategy (8 NeuronCores):
  - Shard the j axis of W.reshape(784, 784, 10) across cores: 98 j's per core.
    Each core reads W/8 + full x (~2.5 MB in fp16); compute is the long pole.
  - Stage 1 (TensorE): U[b, (k,j)] = sum_i x[b,i] * W[i, j_shard, k], x^T tiles
    stationary, W shard moving, accumulating over 7 uniform 112-row i-tiles
    into 8 PSUM banks (4 batch tiles x 2 column halves = 5 k's x 98 j's).
  - Stage 2 (DVE): U * xs multiply (PSUM read) + reduce over j, ~1.14us per
    group; groups must close >=1.2us apart or the DVE backs up.
  - Host: y = sum_c y_part_c + b  (20 KB per core; no collectives needed).

Schedule (calibrated against HW traces):
  - First input chunk completes ~5.6us after kernel start (1.6us framework
    entry + 0.7us queue op + ~3.3us ring latency); total input delivery runs
    at ~0.22-0.3 MB/us; y DMA completion costs a fixed ~1.9us after issue;
    framework teardown is a fixed ~8.5us.  The PE runs at half clock until
    the HAM boost, ~6.3us of GAP-FREE PE activity after the first matmul;
    any PE idle gap resets the ramp (costing ~5us) -- so dummy warmup
    matmuls bridge from t~2.1 until real data lands.
  - xT is laid out bt-major so each batch-tile's stationary set (0.2 MB)
    arrives independently: the first group closes at ~9.5us instead of ~14.
  - Matmul order chases delivery group-major ((bt,h) columns of work), so
    the 8 PSUM groups close every ~1.4us and all but the last DVE stage-2
    hide under the matmul stream.
  - y leaves in 4 group-pair DMAs on the gpsimd ring as pairs complete.
"""

import numpy as np

D = 784
B = 512
C = 10
NCORES = 8
JS = D // NCORES  # 98 j's per core
JK = JS * C  # 980 free columns per core, laid out as (k, j)
HALF = JK // 2  # 490 = 5 k's x 98 j's -> one PSUM bank
KH = C // 2  # 5 k's per half
P = 128
B_TILES = B // P  # 4
IT = 7  # i-tiles
IP = D // IT  # 112 rows per i-tile (uniform, no padding)
N_WARMUP_MM = 10  # dummy matmuls bridging until the first chunks land

MM_DTYPE = "float16"  # dtype of the matmul operands (and their DMA)

_nc_cache = {}


def _build_nc():
    import concourse.bacc as bacc
    import concourse.mybir as mybir
    import concourse.tile as tile

    mm_dt = getattr(mybir.dt, MM_DTYPE)
    f32 = mybir.dt.float32

    nc = bacc.Bacc("TRN2", target_bir_lowering=False)

    # Partition-major DRAM layouts (see _make_in_maps).
    xT = nc.dram_tensor("xT", [IP, B_TILES, IT, P], mm_dt, kind="ExternalInput")
    w = nc.dram_tensor("w", [IP, 2, IT, HALF], mm_dt, kind="ExternalInput")
    xs = nc.dram_tensor("xs", [P, B_TILES, JS], mm_dt, kind="ExternalInput")
    y = nc.dram_tensor("y", [P, 2, B_TILES, KH], f32, kind="ExternalOutput")

    with tile.TileContext(nc) as tc:
        with (
            tc.tile_pool(name="wpool", bufs=8) as wpool,
            tc.tile_pool(name="xpool", bufs=4) as xpool,
            tc.tile_pool(name="xspool", bufs=1) as xspool,
            tc.tile_pool(name="ypool", bufs=1) as ypool,
            tc.tile_pool(name="scratch", bufs=4) as spool,
            tc.tile_pool(name="psum", bufs=8, space="PSUM") as psum_pool,
        ):
            # Dummy warmup operands; memset on gpsimd right after entry so
            # the first warmup matmul (and the HAM ramp clock) starts ASAP.
            dmy_s = spool.tile([IP, P], mm_dt, name="dmy_s", tag="dmy_s", bufs=1)
            dmy_m = spool.tile([IP, HALF], mm_dt, name="dmy_m", tag="dmy_m", bufs=1)
            nc.gpsimd.memset(dmy_s[:], 0.0)
            nc.gpsimd.memset(dmy_m[:], 0.0)

            w_sb = {}  # (it, h) -> [IP, HALF] view

            def w_dma(eng, h, c0, c1):
                wt = wpool.tile(
                    [IP, c1 - c0, HALF], mm_dt, name=f"w_h{h}c{c0}", tag=f"wh{h}"
                )
                eng.dma_start(wt[:], w[:, h, c0:c1, :])
                for it in range(c0, c1):
                    w_sb[(it, h)] = wt[:, it - c0, :]

            xT_sb = {}  # bt -> [IP, IT, P]

            def xt_dma(bt):
                xt = xpool.tile([IP, IT, P], mm_dt, name=f"xt_b{bt}", tag="xt")
                nc.scalar.dma_start(xt[:], xT[:, bt])
                xT_sb[bt] = xt

            # Issue order == per-ring delivery order (queue ops ~650ns each).
            # sync: wh0[0:2], wh0[2:4], wh0[4:7], wh1[4:7]
            # scalar: xt-bt0..3, wh1[0:2], wh1[2:4]
            # gpsimd: xs, then the y pairs (ring stays warm)
            w_dma(nc.sync, 0, 0, 2)
            xt_dma(0)
            xt_dma(1)
            w_dma(nc.sync, 0, 2, 4)
            xt_dma(2)
            w_dma(nc.sync, 0, 4, 7)
            xt_dma(3)
            w_dma(nc.sync, 1, 4, 7)
            w_dma(nc.scalar, 1, 0, 2)
            w_dma(nc.scalar, 1, 2, 4)
            xs_sb = xspool.tile([P, B_TILES, JS], mm_dt)
            nc.gpsimd.dma_start(xs_sb[:], xs[:])

            # PSUM: 8 accumulation groups (bt, h), one bank each.
            pts = {}
            for bt in range(B_TILES):
                for h in range(2):
                    pts[(bt, h)] = psum_pool.tile(
                        [P, HALF], f32, name=f"pt_b{bt}h{h}", tag="pt", bufs=8
                    )

            def warmup(n, bank=(3, 1)):
                # dummy matmuls into a not-yet-opened group's bank; each one
                # opens and closes its own accumulation group.
                for _ in range(n):
                    nc.tensor.matmul(
                        pts[bank][:], dmy_s[:], dmy_m[:], start=True, stop=True
                    )

            warmup(N_WARMUP_MM)

            y_t = ypool.tile([P, 2, B_TILES, KH], f32)

            def mm(it, bt, h):
                nc.tensor.matmul(
                    pts[(bt, h)][:],
                    xT_sb[bt][:, it, :],
                    w_sb[(it, h)][:],
                    start=(it == 0),
                    stop=(it == IT - 1),
                )

            def stage2(bt, h):
                # Multiply on DVE (PSUM read), then reduce over j on DVE.
                pt = pts[(bt, h)]
                scr = spool.tile(
                    [P, HALF], f32, name=f"scr{bt}{h}", tag="scr", bufs=2
                )
                s3 = scr[:].rearrange("p (kh j) -> p kh j", kh=KH)
                p3 = pt[:].rearrange("p (kh j) -> p kh j", kh=KH)
                xs3 = xs_sb[:, bt, None, :].broadcast_to([P, KH, JS])
                nc.vector.tensor_tensor(s3, p3, xs3, mybir.AluOpType.mult)
                nc.vector.tensor_reduce(
                    out=y_t[:, h, bt, :],
                    in_=s3,
                    op=mybir.AluOpType.add,
                    axis=mybir.AxisListType.X,
                )

            def y_dma(h, bt):
                # ship groups (bt-1, h) and (bt, h) together
                nc.gpsimd.dma_start(
                    y[:, h, bt - 1 : bt + 1, :], y_t[:, h, bt - 1 : bt + 1, :]
                )

            # Group-chasing stream: each step is a run of i-tiles for one
            # (bt, h) group, ordered so every run's W/xT chunks have landed
            # and the 8 closures spread ~1.4us apart.
            # "W" entries are single warmup fillers (into g31's still-unopened
            # bank) absorbing DMA arrival jitter without a PE gap/ramp reset.
            SCHED = [
                (0, 0, 0, 2), "W",            # g00 its0-1
                (0, 0, 2, 4), "W",            # g00 its2-3
                (1, 0, 0, 2), (1, 0, 2, 4),   # g10 its0-3
                (0, 0, 4, 7),                 # g00 close c1
                (2, 0, 0, 2),                 # g20 its0-1
                (1, 0, 4, 7),                 # g10 close c2
                (2, 0, 2, 4), (3, 0, 0, 2),   # g20 its2-3, g30 its0-1
                (2, 0, 4, 7),                 # g20 close c3
                (3, 0, 2, 4), (0, 1, 0, 2),   # g30 its2-3, g01 its0-1
                (3, 0, 4, 7),                 # g30 close c4
                (0, 1, 2, 4), (1, 1, 0, 2),   # g01 its2-3, g11 its0-1
                (0, 1, 4, 7),                 # g01 close c5
                (1, 1, 2, 4), (2, 1, 0, 2),   # g11 its2-3, g21 its0-1
                (1, 1, 4, 7),                 # g11 close c6
                (2, 1, 2, 4), (3, 1, 0, 2),   # g21 its2-3, g31 its0-1
                (2, 1, 4, 7),                 # g21 close c7
                (3, 1, 2, 7),                 # g31 its2-6 close c8
            ]
            g31_opened = False
            for step in SCHED:
                if step == "W":
                    if not g31_opened:
                        warmup(1)
                    continue
                bt, h, i0, i1 = step
                if (bt, h) == (3, 1) and i0 == 0:
                    g31_opened = True
                for it in range(i0, i1):
                    mm(it, bt, h)
                if i1 == IT:
                    stage2(bt, h)
                    if bt % 2 == 1:
                        y_dma(h, bt)

    nc.compile()
    return nc


def _get_nc():
    if "nc" not in _nc_cache:
        _nc_cache["nc"] = _build_nc()
    return _nc_cache["nc"]


def _make_in_maps(x, W):
    import concourse.mybir as mybir

    mm_np = mybir.dt.np(getattr(mybir.dt, MM_DTYPE))
    x = np.asarray(x, dtype=np.float32)
    Wr = np.asarray(W, dtype=np.float32).reshape(D, D, C)
    # xT_dram[p, bt, it, q] = x[bt*P + q, it*IP + p]
    xT = np.ascontiguousarray(
        x.T.astype(mm_np).reshape(IT, IP, B_TILES, P).transpose(1, 2, 0, 3)
    )
    in_maps = []
    for c in range(NCORES):
        js, je = c * JS, (c + 1) * JS
        # wsh[i, k*JS + j] = Wr[i, js+j, k]; then [p, h, it, col] partition-major
        wsh = Wr[:, js:je, :].transpose(0, 2, 1).reshape(D, JK).astype(mm_np)
        wshard = np.ascontiguousarray(
            wsh.reshape(IT, IP, 2, HALF).transpose(1, 2, 0, 3)
        )
        # xs_dram[p, bt, j] = x[bt*P + p, js + j]
        xsl = np.ascontiguousarray(
            x[:, js:je].reshape(B_TILES, P, JS).transpose(1, 0, 2).astype(mm_np)
        )
        in_maps.append({"xT": xT, "w": wshard, "xs": xsl})
    return in_maps


def run_spmd(x, W, **spmd_kwargs):
    """Compile/run the SPMD kernel; returns (partials, BassKernelResults)."""
    from concourse.bass_utils import run_bass_kernel_spmd

    nc = _get_nc()
    in_maps = _make_in_maps(x, W)
    res = run_bass_kernel_spmd(nc, in_maps, core_ids=list(range(NCORES)), **spmd_kwargs)
    # y_dram[p, h, bt, kh] -> y[bt*P + p, h*KH + kh]
    partials = [
        r["y"].transpose(2, 0, 1, 3).reshape(B_TILES, P, C).reshape(B, C)
        for r in res.results
    ]
    return partials, res


def kernel(x, W, b):
    partials, _ = run_spmd(x, W)
    y = np.sum(np.stack(partials, 0), axis=0, dtype=np.float64) + np.asarray(
        b, dtype=np.float64
    )
    return y.astype(np.float32)
